# revision 5
# baseline (speedup 1.0000x reference)
"""LIF (leaky integrate-and-fire) spiking recurrence on 8 Trainium2 cores.

Full input x: [T*bs, C, H, W] = [256, 128, 32, 32] f32 with T=8, bs=32.
Recurrence over T only, elementwise elsewhere:
    u_t = TAU * u_{t-1} * (1 - (u_{t-1} > VTH)) + x_t ;  o_t = (u_t > VTH)

Sharding: fully data-parallel over batch (bs=32 -> 4 per core), no collectives.
Each core views its [4,128,32,32] per-timestep slab as a flat
[128 partitions, 4096] tile.

Two structural tricks vs the naive f32 version:

1. Byte spikes: the output is pure 0/1, so ACT computes
   s = sign(v_t - 2^t) -> uint8 in a single pass and the host maps
   byte==1 -> 1.0f (exact for any float->u8 conversion semantics).
   Store traffic drops 4x: 16.78 MB -> 4.19 MB per core.

2. Scaled state + DMA-accumulate: with v_t = 2^t * u_t (host pre-scales
   X_t = 2^t * x_t, exact power-of-two scaling, recurrence stays bitwise
   identical), the update becomes
       w_t = v_t * (v_t <= 2^t)        (one DVE pass)
       v_{t+1} = w_t + X_{t+1}         (CCE ADD inside the SWDGE DMA!)
   so the add happens inside the DMA engine during the load and DVE does
   ONE pass per step instead of two. x is never SBUF-resident; two
   ping-pong tiles hold the state.

Engine budget per core: DMA 16.78+4.19 MB ~ 61 us (bottleneck),
DVE 7 masks ~ 31 us, ACT 8 signs ~ 30 us.
"""

import numpy as np

import concourse.tile as tile
from concourse import bacc, mybir
from concourse.bass_utils import run_bass_kernel_spmd

T = 8
BS = 32
C = 128
HW = 32 * 32
NCORES = 8
BSH = BS // NCORES          # 4 batch elements per core
P = 128                     # SBUF partitions
FREE = BSH * C * HW // P    # 4096 f32 per partition per timestep
CH = 4                      # chunks per timestep
CHF = FREE // CH            # 1024
VTH = 1.0
F32 = mybir.dt.float32
U8 = mybir.dt.uint8

_nc_cache = None


def _build():
    nc = bacc.Bacc("TRN2", target_bir_lowering=False, debug=False, num_devices=NCORES)
    x_d = nc.dram_tensor("x", [T, P, FREE], F32, kind="ExternalInput").ap()
    o_d = nc.dram_tensor("o", [T, P, FREE], U8, kind="ExternalOutput").ap()

    with tile.TileContext(nc) as tc:
        with (
            tc.tile_pool(name="vp", bufs=1) as vp,
            tc.tile_pool(name="op", bufs=2) as op,
            tc.tile_pool(name="cp", bufs=1) as cp,
        ):
            # Per-partition bias constants -2^t for the ACT sign pass.
            nbias = cp.tile([P, T], F32)
            for t in range(T):
                nc.vector.memset(nbias[:, t:t + 1], -float(2 ** t))

            xv = x_d.rearrange("t p f -> p t f")  # [128, T, FREE] HBM view

            # Ping-pong state tiles; v_t alternates between them.
            va = vp.tile([P, FREE], F32, name="va")
            vb = vp.tile([P, FREE], F32, name="vb")
            tiles = [va, vb]

            # Initial plain load of X_0 on the HWDGE (sync) ring, chunked so
            # step 0 compute starts as soon as the first chunk lands.
            for c in range(CH):
                fsl = slice(c * CHF, (c + 1) * CHF)
                nc.sync.dma_start(out=va[:, fsl], in_=xv[:, 0, fsl])

            for t in range(T):
                v = tiles[t % 2]
                w = tiles[(t + 1) % 2]
                o = op.tile([P, FREE], U8, name="o", tag="o")
                for c in range(CH):
                    fsl = slice(c * CHF, (c + 1) * CHF)
                    # s = sign(v - 2^t): +1 iff spike. uint8 store; host
                    # decodes byte==1 (exact whatever -1.0->u8 does).
                    nc.scalar.activation(
                        o[:, fsl], v[:, fsl], mybir.ActivationFunctionType.Sign,
                        bias=nbias[:, t:t + 1], scale=1.0,
                    )
                    if t < T - 1:
                        # w = v * (v <= 2^t)  (zero the spiked neurons)
                        nc.vector.scalar_tensor_tensor(
                            w[:, fsl], v[:, fsl], float(2 ** t), v[:, fsl],
                            op0=mybir.AluOpType.is_le, op1=mybir.AluOpType.mult,
                        )
                        # v_{t+1} = w + X_{t+1}: CCE ADD inside the SWDGE DMA.
                        nc.gpsimd.dma_start(
                            out=w[:, fsl], in_=xv[:, t + 1, fsl],
                            accum_op=mybir.AluOpType.add,
                        )
                    # Spike stores ride the HWDGE (sync) ring; loads there
                    # finished after X_0, so no FIFO blocking.
                    nc.sync.dma_start(out=o_d[t][:, fsl], in_=o[:, fsl])

    nc.compile()
    return nc


def _get_nc():
    global _nc_cache
    if _nc_cache is None:
        _nc_cache = _build()
    return _nc_cache


def _run(x: np.ndarray, **spmd_kwargs):
    nc = _get_nc()
    xr = np.asarray(x, dtype=np.float32).reshape(T, BS, C, HW)
    # Pre-scale X_t = 2^t * x_t (exact power-of-two scaling).
    scale = (2.0 ** np.arange(T, dtype=np.float32)).reshape(T, 1, 1, 1)
    xs = xr * scale
    in_maps = [
        {"x": np.ascontiguousarray(xs[:, k * BSH:(k + 1) * BSH]).reshape(T, P, FREE)}
        for k in range(NCORES)
    ]
    res = run_bass_kernel_spmd(nc, in_maps, core_ids=list(range(NCORES)), **spmd_kwargs)
    out = np.empty((T, BS, C, HW), dtype=np.float32)
    for k in range(NCORES):
        ok = res.results[k]["o"].reshape(T, BSH, C, HW)
        out[:, k * BSH:(k + 1) * BSH] = (ok == 1)
    return out.reshape(T * BS, C, 32, 32), res


def kernel(x: np.ndarray) -> np.ndarray:
    out, _ = _run(x)
    return out
